# revision 2
# baseline (speedup 1.0000x reference)
"""Haar DWT on 8 Trainium2 NeuronCores (batch-parallel, 1 image per core).

Layout: partition p of tile t holds 8 consecutive input rows (4 output
row-pairs) of one channel: global row-block g = 128*t + p, channel
c = g//64, rows 8*(g%64)..+8. Free dim = 4096 (8 rows x 512 cols).

Per-core pipeline, 32 tiles (2 channels each):
  1. in-DMA: 2 MiB fully contiguous, 16 KiB per-partition descriptors
     (SP HWDGE ring)
  2. DVE stage 1 (column butterfly, stride-2 views, FD=2048 each) with
     the 0.5 Haar scale folded in via tensor_tensor_reduce
     (out = (a op b) * 0.5; the [P,1] accum_out is a dummy):
       sum1 = (x[0::2] + x[1::2]) * 0.5, diff1 = (x[1::2] - x[0::2]) * 0.5
     sd layout per partition: [sum|diff][j:4 row-pairs][parity:2][w:256]
  3. stage 2 (row butterfly, 3-dim APs, FD=2048), split across engines:
       DVE    add -> LL (from sum) + HL (from diff)
       GpSimd sub -> LH (from sum) + HH (from diff)
     o_sb layout [sb:4][j:4][w:256]: per partition each subband block is
     4 KiB = 4 consecutive output rows, contiguous in DRAM
  4. two 1 MiB out-DMAs per tile ({LL,HL} after the DVE add, {LH,HH}
     after the GpSimd sub; 3-dim APs, 4 KiB per-partition descriptors)
     on the ACT HWDGE ring, which now carries ONLY stores (no scalar
     mul in front), so a stalled store never blocks later tiles' work.

Engine budget per core (32 tiles): DVE 3 ops/tile ~ 220 us, GpSimd
1 op/tile ~ 140 us, ScalarE/PE idle, DMA ~334 us/engine busy -> DMA
bound at the descriptor line rate.
"""

import sys

sys.path.insert(0, "/opt/trn_rl_repo")

import numpy as np

import concourse.bass as bass
import concourse.bacc as bacc
import concourse.mybir as mybir
from concourse import tile
from concourse.bass_utils import run_bass_kernel_spmd

N_CORES = 8
C = 64
H = 512
W = 512
HO = H // 2
WO = W // 2
P = 128
FD = 4096               # 8 input rows per partition
TILES = C * H * W // (P * FD)  # 32
OFD = FD // 4           # 1024: out elems per partition per subband

F32 = mybir.dt.float32


def build_nc() -> bass.Bass:
    nc = bacc.Bacc()
    x = nc.dram_tensor("x", [C, H, W], F32, kind="ExternalInput")
    out = nc.dram_tensor("out", [4 * C, HO, WO], F32, kind="ExternalOutput")

    # [4096 row-blocks, 4096]: row-block g = (c, hb), free = (r, w), h = 8*hb + r
    x_v = x.rearrange("c (hb r) w -> (c hb) (r w)", r=8)
    # per subband: out[sb*64 + cc, h, w] flattened — offset = g*1024 + j*256 + w
    out_v = out.rearrange("(s cc) h w -> s (cc h w)", s=4)

    add = mybir.AluOpType.add
    sub = mybir.AluOpType.subtract
    amax = mybir.AluOpType.max

    with tile.TileContext(nc) as tc:
        with (
            tc.tile_pool(name="pin", bufs=5) as pin,
            tc.tile_pool(name="psd", bufs=3) as psd,
            tc.tile_pool(name="pout", bufs=3) as pout,
            tc.tile_pool(name="pacc", bufs=3) as pacc,
        ):
            for t in range(TILES):
                in_sb = pin.tile([P, FD], F32)
                nc.sync.dma_start(in_sb[:], x_v[t * P : (t + 1) * P, :])

                sd = psd.tile([P, FD], F32)
                acc = pacc.tile([P, 2], F32)
                i3 = in_sb[:].rearrange("p (k two) -> p k two", two=2)
                # stage 1 with the 0.5 Haar scale folded in:
                # out = (in0 op0 in1) * 0.5; accum_out (max-reduce) unused
                nc.vector.tensor_tensor_reduce(
                    out=sd[:, 0:2048], in0=i3[:, :, 0], in1=i3[:, :, 1],
                    scale=0.5, scalar=0.0, op0=add, op1=amax,
                    accum_out=acc[:, 0:1],
                )
                nc.vector.tensor_tensor_reduce(
                    out=sd[:, 2048:4096], in0=i3[:, :, 1], in1=i3[:, :, 0],
                    scale=0.5, scalar=0.0, op0=sub, op1=amax,
                    accum_out=acc[:, 1:2],
                )

                o_sb = pout.tile([P, FD], F32)
                # sd: [half2][j:4][parity:2][w:256]; o_sb: [sb:4][j:4][w:256]
                s4 = sd[:].rearrange(
                    "p (half j parity w) -> p half j parity w",
                    half=2, j=4, parity=2,
                )
                o4 = o_sb[:].rearrange("p (sb j w) -> p sb j w", sb=4, j=4)
                # LL (sb0) from sum-half, HL (sb2) from diff-half — DVE
                nc.vector.tensor_add(
                    o4[:, 0::2, :, :], s4[:, :, :, 0, :], s4[:, :, :, 1, :]
                )
                # LH (sb1) from sum-half, HH (sb3) from diff-half — GpSimd
                nc.gpsimd.tensor_sub(
                    o4[:, 1::2, :, :], s4[:, :, :, 1, :], s4[:, :, :, 0, :]
                )

                # two 1 MiB stores per tile (ACT ring): {LL,HL} can go as
                # soon as the DVE add lands, {LH,HH} after the GpSimd sub.
                dst = out_v[:, t * P * OFD : (t + 1) * P * OFD].rearrange(
                    "s (p f) -> p s f", f=OFD
                )
                src4 = o_sb[:].rearrange("p (s f) -> p s f", s=4)
                nc.scalar.dma_start(dst[:, 0::2, :], src4[:, 0::2, :])
                nc.scalar.dma_start(dst[:, 1::2, :], src4[:, 1::2, :])

    nc.finalize()
    return nc


_NC_CACHE: dict = {}


def _get_nc() -> bass.Bass:
    if "nc" not in _NC_CACHE:
        _NC_CACHE["nc"] = build_nc()
    return _NC_CACHE["nc"]


def kernel(x: np.ndarray) -> np.ndarray:
    x = np.asarray(x)
    assert x.shape == (N_CORES, C, H, W), x.shape
    nc = _get_nc()
    in_maps = [{"x": np.ascontiguousarray(x[i])} for i in range(N_CORES)]
    res = run_bass_kernel_spmd(nc, in_maps, list(range(N_CORES)))
    return np.stack([res.results[i]["out"] for i in range(N_CORES)], axis=0)


# revision 3
# speedup vs baseline: 1.0066x; 1.0066x over previous
"""Haar DWT on 8 Trainium2 NeuronCores (batch-parallel, 1 image per core).

Layout: partition p of tile t holds 8 consecutive input rows (4 output
row-pairs) of one channel: global row-block g = 128*t + p, channel
c = g//64, rows 8*(g%64)..+8. Free dim = 4096 (8 rows x 512 cols).

Per-core pipeline, 32 tiles (2 channels each):
  1. in-DMA: 2 MiB fully contiguous, 16 KiB per-partition descriptors
     (SP HWDGE ring)
  2. DVE stage 1 (column butterfly, stride-2 views, FD=2048 each) with
     the 0.5 Haar scale folded in via tensor_tensor_reduce
     (out = (a op b) * 0.5; the [P,1] accum_out is a dummy):
       sum1 = (x[0::2] + x[1::2]) * 0.5, diff1 = (x[1::2] - x[0::2]) * 0.5
     sd layout per partition: [sum|diff][j:4 row-pairs][parity:2][w:256]
  3. stage 2 (row butterfly, 3-dim APs, FD=2048), split across engines:
       DVE    add -> LL (from sum) + HL (from diff)
       GpSimd sub -> LH (from sum) + HH (from diff)
     o_sb layout [sb:4][j:4][w:256]: per partition each subband block is
     4 KiB = 4 consecutive output rows, contiguous in DRAM
  4. two 1 MiB out-DMAs per tile ({LL,HL} after the DVE add, {LH,HH}
     after the GpSimd sub; 3-dim APs, 4 KiB per-partition descriptors)
     on the ACT HWDGE ring, which now carries ONLY stores (no scalar
     mul in front), so a stalled store never blocks later tiles' work.

Engine budget per core (32 tiles): DVE 3 ops/tile ~ 220 us, GpSimd
1 op/tile ~ 140 us, ScalarE/PE idle, DMA ~334 us/engine busy -> DMA
bound at the descriptor line rate.
"""

import sys

sys.path.insert(0, "/opt/trn_rl_repo")

import numpy as np

import concourse.bass as bass
import concourse.bacc as bacc
import concourse.mybir as mybir
from concourse import tile
from concourse.bass_utils import run_bass_kernel_spmd

N_CORES = 8
C = 64
H = 512
W = 512
HO = H // 2
WO = W // 2
P = 128
FD = 4096               # 8 input rows per partition
TILES = C * H * W // (P * FD)  # 32
OFD = FD // 4           # 1024: out elems per partition per subband

F32 = mybir.dt.float32


def build_nc() -> bass.Bass:
    nc = bacc.Bacc()
    x = nc.dram_tensor("x", [C, H, W], F32, kind="ExternalInput")
    out = nc.dram_tensor("out", [4 * C, HO, WO], F32, kind="ExternalOutput")

    # [4096 row-blocks, 4096]: row-block g = (c, hb), free = (r, w), h = 8*hb + r
    x_v = x.rearrange("c (hb r) w -> (c hb) (r w)", r=8)
    # per subband: out[sb*64 + cc, h, w] flattened — offset = g*1024 + j*256 + w
    out_v = out.rearrange("(s cc) h w -> s (cc h w)", s=4)

    add = mybir.AluOpType.add
    sub = mybir.AluOpType.subtract
    amax = mybir.AluOpType.max

    with tile.TileContext(nc) as tc:
        with (
            tc.tile_pool(name="pin", bufs=5) as pin,
            tc.tile_pool(name="psd", bufs=3) as psd,
            tc.tile_pool(name="pout", bufs=3) as pout,
        ):
            for t in range(TILES):
                in_sb = pin.tile([P, FD], F32)
                nc.sync.dma_start(in_sb[:], x_v[t * P : (t + 1) * P, :])

                nc.scalar.mul(in_sb[:], in_sb[:], 0.5)

                sd = psd.tile([P, FD], F32)
                i3 = in_sb[:].rearrange("p (k two) -> p k two", two=2)
                nc.vector.tensor_add(sd[:, 0:2048], i3[:, :, 0], i3[:, :, 1])
                nc.vector.tensor_sub(sd[:, 2048:4096], i3[:, :, 1], i3[:, :, 0])

                o_sb = pout.tile([P, FD], F32)
                # sd: [half2][j:4][parity:2][w:256]; o_sb: [sb:4][j:4][w:256]
                s4 = sd[:].rearrange(
                    "p (half j parity w) -> p half j parity w",
                    half=2, j=4, parity=2,
                )
                o4 = o_sb[:].rearrange("p (sb j w) -> p sb j w", sb=4, j=4)
                # LL (sb0) from sum-half, HL (sb2) from diff-half — DVE
                nc.vector.tensor_add(
                    o4[:, 0::2, :, :], s4[:, :, :, 0, :], s4[:, :, :, 1, :]
                )
                # LH (sb1) from sum-half, HH (sb3) from diff-half
                nc.vector.tensor_sub(
                    o4[:, 1::2, :, :], s4[:, :, :, 1, :], s4[:, :, :, 0, :]
                )

                # two 1 MiB stores per tile (ACT ring): {LL,HL} can go as
                # soon as the DVE add lands, {LH,HH} after the GpSimd sub.
                dst = out_v[:, t * P * OFD : (t + 1) * P * OFD].rearrange(
                    "s (p f) -> p s f", f=OFD
                )
                src4 = o_sb[:].rearrange("p (s f) -> p s f", s=4)
                nc.scalar.dma_start(dst[:, 0::2, :], src4[:, 0::2, :])
                nc.scalar.dma_start(dst[:, 1::2, :], src4[:, 1::2, :])

    nc.finalize()
    return nc


_NC_CACHE: dict = {}


def _get_nc() -> bass.Bass:
    if "nc" not in _NC_CACHE:
        _NC_CACHE["nc"] = build_nc()
    return _NC_CACHE["nc"]


def kernel(x: np.ndarray) -> np.ndarray:
    x = np.asarray(x)
    assert x.shape == (N_CORES, C, H, W), x.shape
    nc = _get_nc()
    in_maps = [{"x": np.ascontiguousarray(x[i])} for i in range(N_CORES)]
    res = run_bass_kernel_spmd(nc, in_maps, list(range(N_CORES)))
    return np.stack([res.results[i]["out"] for i in range(N_CORES)], axis=0)


# revision 4
# speedup vs baseline: 1.1695x; 1.1618x over previous
"""Haar DWT on 8 Trainium2 NeuronCores (batch-parallel, 1 image per core).

Layout: partition p of tile t holds 16 consecutive input rows (8 output
rows) of one channel: 16-row block g2 = 128*t + p, channel c = g2//32,
rows 16*(g2%32)..+16. Tile free dim = 8192 (16 rows x 512 cols), 4 MiB.

Fewer, bigger DMAs than the 2 MiB/tile variant: per-dma completion
semaphore descriptors cost ~500 ns (loads) / ~110 ns (stores) on every
SDMA engine, so halving the dma_start count reclaims ~15 us of
per-engine DMA busy time. Store descriptors double to 8 KiB (8
consecutive output rows per partition per subband).

Per-core pipeline, 16 tiles (4 channels each), software-pipelined
(loads issued 2 tiles ahead, the ScalarE 0.5 mul 1 tile ahead, so the
ACT queue never head-of-line-blocks a ready mul behind a store that is
still waiting on DVE):
  1. in-DMA: 4 MiB fully contiguous, 32 KiB per-partition descriptors
     (SP HWDGE ring)
  2. ScalarE in-place x *= 0.5 (exact in fp32; folds the Haar scale)
  3. compute in two 4096-elem halves (8 rows each) to keep the sd pool
     small; per half:
       DVE stage 1 (column butterfly, stride-2 views, FD=2048):
         sum1 = x[0::2] + x[1::2], diff1 = x[1::2] - x[0::2]
       DVE stage 2 (row butterfly, FD=2048):
         add -> LL + HL, sub -> LH + HH
     (all-DVE: concurrent GpSimd tensor ops contend with DVE on the
     shared SBUF port pair — measured 2.2 -> 6.5 us per op — and
     tensor_tensor_reduce, which would fold the 0.5 scale into stage 1,
     fails at runtime on HW, so ScalarE keeps the scale pass)
     o_sb layout [sb:4][j:8][w:256]: per partition each subband block
     is 8 KiB = 8 consecutive output rows, contiguous in DRAM
  4. two 2 MiB out-DMAs per tile ({LL,HL} / {LH,HH}; 3-dim APs, 8 KiB
     per-partition descriptors) on the ACT HWDGE ring

Engine busy per core: DVE ~283 us, ScalarE ~58 us, DMA ~320 us/engine
busy -> DMA bound at the descriptor line rate.
"""

import sys

sys.path.insert(0, "/opt/trn_rl_repo")

import numpy as np

import concourse.bass as bass
import concourse.bacc as bacc
import concourse.mybir as mybir
from concourse import tile
from concourse.bass_utils import run_bass_kernel_spmd

N_CORES = 8
C = 64
H = 512
W = 512
HO = H // 2
WO = W // 2
P = 128
FD = 8192                      # 16 input rows per partition
TILES = C * H * W // (P * FD)  # 16
OFD = FD // 4                  # 2048: out elems per partition per subband
HFD = FD // 2                  # 4096: half-tile free dim

F32 = mybir.dt.float32


def build_nc() -> bass.Bass:
    nc = bacc.Bacc()
    x = nc.dram_tensor("x", [C, H, W], F32, kind="ExternalInput")
    out = nc.dram_tensor("out", [4 * C, HO, WO], F32, kind="ExternalOutput")

    # [2048 row-blocks, 8192]: block g2 = (c, hb), free = (r:16, w:512)
    x_v = x.rearrange("c (hb r) w -> (c hb) (r w)", r=16)
    # per subband: out[sb*64 + cc, h, w] flattened — block g2 owns the
    # contiguous 2048-elem range starting at g2*2048
    out_v = out.rearrange("(s cc) h w -> s (cc h w)", s=4)

    with tile.TileContext(nc) as tc:
        with (
            tc.tile_pool(name="pin", bufs=3) as pin,
            tc.tile_pool(name="psd", bufs=2) as psd,
            tc.tile_pool(name="pout", bufs=2) as pout,
        ):
            ins: dict[int, object] = {}

            def issue_load(t):
                in_sb = pin.tile([P, FD], F32)
                ins[t] = in_sb
                nc.sync.dma_start(in_sb[:], x_v[t * P : (t + 1) * P, :])

            def issue_scale(t):
                in_sb = ins[t]
                nc.scalar.mul(in_sb[:], in_sb[:], 0.5)

            def issue_rest(t):
                in_sb = ins.pop(t)
                o_sb = pout.tile([P, FD], F32)
                o4 = o_sb[:].rearrange("p (sb j w) -> p sb j w", sb=4, j=8)
                for h in range(2):
                    sd = psd.tile([P, HFD], F32)
                    i3 = in_sb[:, h * HFD : (h + 1) * HFD].rearrange(
                        "p (k two) -> p k two", two=2
                    )
                    nc.vector.tensor_add(sd[:, 0:2048], i3[:, :, 0], i3[:, :, 1])
                    nc.vector.tensor_sub(sd[:, 2048:4096], i3[:, :, 1], i3[:, :, 0])

                    # sd: [half2][j:4][parity:2][w:256]
                    s4 = sd[:].rearrange(
                        "p (half j parity w) -> p half j parity w",
                        half=2, j=4, parity=2,
                    )
                    o4h = o4[:, :, 4 * h : 4 * h + 4, :]
                    # LL (sb0) from sum-half, HL (sb2) from diff-half
                    nc.vector.tensor_add(
                        o4h[:, 0::2, :, :], s4[:, :, :, 0, :], s4[:, :, :, 1, :]
                    )
                    # LH (sb1) from sum-half, HH (sb3) from diff-half
                    nc.vector.tensor_sub(
                        o4h[:, 1::2, :, :], s4[:, :, :, 1, :], s4[:, :, :, 0, :]
                    )

                # two 2 MiB stores per tile (ACT ring)
                dst = out_v[:, t * P * OFD : (t + 1) * P * OFD].rearrange(
                    "s (p f) -> p s f", f=OFD
                )
                src4 = o_sb[:].rearrange("p (s f) -> p s f", s=4)
                nc.scalar.dma_start(dst[:, 0::2, :], src4[:, 0::2, :])
                nc.scalar.dma_start(dst[:, 1::2, :], src4[:, 1::2, :])

            issue_load(0)
            issue_load(1)
            issue_scale(0)
            for t in range(TILES):
                if t + 2 < TILES:
                    issue_load(t + 2)
                if t + 1 < TILES:
                    issue_scale(t + 1)
                issue_rest(t)

    nc.finalize()
    return nc


_NC_CACHE: dict = {}


def _get_nc() -> bass.Bass:
    if "nc" not in _NC_CACHE:
        _NC_CACHE["nc"] = build_nc()
    return _NC_CACHE["nc"]


def kernel(x: np.ndarray) -> np.ndarray:
    x = np.asarray(x)
    assert x.shape == (N_CORES, C, H, W), x.shape
    nc = _get_nc()
    in_maps = [{"x": np.ascontiguousarray(x[i])} for i in range(N_CORES)]
    res = run_bass_kernel_spmd(nc, in_maps, list(range(N_CORES)))
    return np.stack([res.results[i]["out"] for i in range(N_CORES)], axis=0)
